# revision 11
# baseline (speedup 1.0000x reference)
"""Trainium2 Bass kernel for the sparse-attention (local 3x3 unfold) problem.

Math (per batch-channel (b,c), H=W=128, K=3, pad=1):
  ku = unfold(key)  -> [9, L] raw-flat, reinterpreted [L, 9]
  qu = unfold(query)
  out1 = ku * qu[:, 4:5] ; out2 = ku[:, 4:5] * qu   (as [L, 9] views)

The flat per-channel output index n in [0, 9L) decomposes two ways:
  * n = 128*q + j           (chunk q = one (patch p2=q//128, row i2=q%128)
                             slice: 128 contiguous floats of a dj-shifted,
                             row-padded image variant)
  * n = 9*g + e             (group g shares one stride-9 "center" factor)

Device layout (v2, "fat rows"): channel ch of a tile owns 16 partitions
(r = 16*ch + rr) with FREE = 9216 = 72 chunks per partition, n = 9216*rr + f.
  * FREE % 9 == 0 keeps the stride-9 center-broadcast multiply phase-free
    on every partition (one DVE op covers all 8 channels of a tile).
  * Loads: the (72-chunk partition) x (128-chunk patch) overlap gives 24
    maximal segments per channel; each is ONE contiguous DRAM run of the
    variant image -> one descriptor (2-18 KiB) per (segment, channel).
  * Stores: per-channel DRAM is contiguous with offset r*FREE uniform in
    the partition index -> one dma_start moves a whole tile half
    (128 descriptors x 9 KiB).

dtype: fp16 end-to-end on device (harness tolerance 2e-2 vs ~1.5e-3 fp16
error); host upcasts to fp32.  Halves both HBM read and write traffic.

Sharding: pure data-parallel over the 256 (b,c) channels; 32 per core.
"""

import sys

for _p in ("/opt/trn_rl_repo", "/opt/pypackages"):
    if _p not in sys.path:
        sys.path.insert(0, _p)

import numpy as np

import concourse.bass as bass
import concourse.mybir as mybir
import concourse.tile as tile
from concourse.bass import AP
from concourse.bass_utils import run_bass_kernel_spmd
from concourse.vector_clock import ScopedClock

# ---------------------------------------------------------------------------
# Patch: this container's walrus rejects >1 sync-wait on the Tile tail Drain
# ("Too many sync wait commands").  Spill extra waits onto SP NOPs, which
# execute in program order before the all-engine barrier, preserving the
# "all work done before sem clear" semantics.
# ---------------------------------------------------------------------------


def _drain_and_barrier(self, tick_clock, wait_clock):
    nc = self.nc
    drain_inst = nc.sync.drain()
    wait_clock.add_sem_waits(
        drain_inst.ins, ScopedClock({None: tick_clock.global_clock})
    )
    si = drain_inst.ins.sync_info
    if si is not None and len(si.on_wait) > 1:
        waits = list(si.on_wait)
        drain_inst.ins.sync_info = mybir.SyncInfo(
            on_wait=waits[:1], on_update=list(si.on_update)
        )
        for w in waits[1:]:
            nop = nc.sync.nop(nofuse=True)
            nop.ins.sync_info = mybir.SyncInfo(on_wait=[w], on_update=[])

    nc.all_engine_barrier()
    assert self.sems is not None
    popped = nc._tile_sem_poison_stack.pop()
    assert popped is self._sem_poison
    nc.clear_and_free_semaphores(list(self.sems.allocated().values()))
    nc.all_engine_barrier()


tile.TileContext._drain_and_barrier = _drain_and_barrier


def _split_waits(nc, maxw=1):
    """Walrus here allows only `maxw` sync-waits per instruction: move extra
    waits onto same-engine NOPs inserted immediately before the instruction
    (same engine stream => executes before it)."""
    for fn in nc.m.functions:
        for bb in fn.blocks:
            out = []
            for inst in bb.instructions:
                si = getattr(inst, "sync_info", None)
                if si is not None and len(si.on_wait) > maxw:
                    waits = list(si.on_wait)
                    for w in waits[:-maxw]:
                        nop = mybir.InstNoOp(
                            name=nc.get_next_instruction_name(),
                            bass_nofuse=True,
                        )
                        nop.engine = inst.engine
                        nop.sync_info = mybir.SyncInfo(on_wait=[w], on_update=[])
                        nc.register_instruction(nop)
                        out.append(nop)
                    inst.sync_info = mybir.SyncInfo(
                        on_wait=waits[-maxw:], on_update=list(si.on_update)
                    )
                out.append(inst)
            bb.instructions[:] = out

# ---------------------------------------------------------------------------

F16 = mybir.dt.float16

N_CORES = 8
B, C, H, W = 4, 64, 128, 128
BC = B * C                # 256 channels
CPC = BC // N_CORES       # 32 channels per core
NCH = 16                  # channels per tile (x8 partitions = 128)
NG = CPC // NCH           # channel groups per core
HP = H + 2                # padded rows
VAR = HP * W              # one dj-variant: [130, 128]
IMG = 3 * VAR             # three dj-variants per channel
L = H * W
PPCH = 8                  # partitions per channel
CHF = 18432               # elements per partition per channel (144 chunks)
NT = 8                    # f-sub-tiles per channel group
TCH = 144 // NT           # chunks per sub-tile per partition
FREE = CHF // NT          # tile free width (= 9 * k, phase-free multiply)
OUT_CH = 9 * L            # 147456 = PPCH * CHF
assert FREE % 9 == 0


def _split(q0, q1, extra=()):
    """Maximal runs of [q0,q1) not crossing sub-tile (TCH), patch (128),
    or `extra` boundaries."""
    bounds = sorted(
        {q0, q1}
        | {q for q in range(0, 1153, TCH) if q0 < q < q1}
        | {q for q in range(0, 1153, 128) if q0 < q < q1}
        | {q for q in extra if q0 < q < q1}
    )
    return list(zip(bounds[:-1], bounds[1:]))


def _dst(qs):
    """(partition rr, f offset in CHF) of chunk qs."""
    rr = qs // 144
    return rr, (qs - 144 * rr) * 128


def _hbm_segments():
    """HBM-loaded q-runs: the di=1 patches (q in [384,768)) plus the six
    zero-pad rows that di=0/di=2 patches can't copy from di=1.  Per run:
    (rr, f_off, len, src_off)."""
    runs = [(0, 1), (128, 129), (256, 257), (384, 768),
            (895, 896), (1023, 1024), (1151, 1152)]
    segs = []
    for q0, q1 in runs:
        for qs, qe in _split(q0, q1):
            rr, f_off = _dst(qs)
            p2 = qs // 128
            di, dj = divmod(p2, 3)
            segs.append(
                (rr, f_off, (qe - qs) * 128,
                 dj * VAR + (qs - 128 * p2 + di) * W)
            )
    return segs


def _s2s_segments():
    """SBUF->SBUF duplication: chunk q of a di=0 (di=2) patch equals chunk
    q+383 (q-383) of the di=1 patch with the same dj.  Runs split where
    the SOURCE crosses a partition.  Per run: (rr, f_off, len, rr2, f2)."""
    runs = [(1, 128, 383), (129, 256, 383), (257, 384, 383),
            (768, 895, -383), (896, 1023, -383), (1024, 1151, -383)]
    segs = []
    for q0, q1, dq in runs:
        extra = [q for q in range(q0, q1) if (q + dq) % 144 == 0]
        for qs, qe in _split(q0, q1, extra):
            rr, f_off = _dst(qs)
            rr2, f2 = _dst(qs + dq)
            segs.append((rr, f_off, (qe - qs) * 128, rr2, f2))
    return segs


_HBM_SEGS = _hbm_segments()
_S2S_SEGS = _s2s_segments()


def _build_program():
    nc = bass.Bass(trn_type="TRN2")
    kp = nc.dram_tensor("kp", [CPC, 3, HP, W], F16, kind="ExternalInput")
    qp = nc.dram_tensor("qp", [CPC, 3, HP, W], F16, kind="ExternalInput")
    o1 = nc.dram_tensor("o1", [CPC, OUT_CH], F16, kind="ExternalOutput")
    o2 = nc.dram_tensor("o2", [CPC, OUT_CH], F16, kind="ExternalOutput")

    # Three dynamic DMA queues (SP-HWDGE, ACT-HWDGE, Pool-SWDGE); strict
    # round-robin keeps every queue fed (prior HW finding: greedy
    # bin-packing clusters DMAs per queue and the per-engine FIFO then
    # serializes them).
    engines = [nc.sync, nc.scalar, nc.gpsimd]
    eng_i = [0]

    def eng():
        e = engines[eng_i[0] % len(engines)]
        eng_i[0] += 1
        return e

    def do_loads(g, tk, tq):
        # 16 descriptors (one per channel) per dma_start, mutually
        # non-contiguous in stream order.  Descriptors are dealt to
        # SDMA-engine slots round-robin from slot 0 and consecutive
        # contiguous descriptors re-aggregate into one packet, so
        # 8-descriptor loads pile onto engines 0-7 (HW-measured: 86%
        # busy vs 39% on engines 8-15); 16 channel-major descriptors
        # engage all 16.
        for srcd, tt in ((kp, tk), (qp, tq)):
            th = tt[:].tensor
            for rr, f_off, seg_len, src_off in _HBM_SEGS:
                eng().dma_start(
                    AP(th, rr * CHF + f_off, [[PPCH * CHF, NCH], [1, seg_len]]),
                    AP(srcd, g * NCH * IMG + src_off, [[IMG, NCH], [1, seg_len]]),
                )

    def do_s2s(tk, tq):
        # On-chip duplication of the di=0/di=2 patches out of the resident
        # di=1 rows: moves 2/3 of the input expansion off HBM and onto the
        # SBUF fabric.
        for tt in (tk, tq):
            th = tt[:].tensor
            for rr, f_off, seg_len, rr2, f2 in _S2S_SEGS:
                eng().dma_start(
                    AP(th, rr * CHF + f_off, [[PPCH * CHF, NCH], [1, seg_len]]),
                    AP(th, rr2 * CHF + f2, [[PPCH * CHF, NCH], [1, seg_len]]),
                )

    def do_mul_store(g, t, tk, tq, o1t, o2t):
        tkh, tqh = tk[:].tensor, tq[:].tensor
        ap_d = [[CHF, 128], [9, FREE // 9], [1, 9]]
        ap_b = [[CHF, 128], [9, FREE // 9], [0, 9]]
        for (od, ot, full, cen) in (
            (o1, o1t, tkh, tqh),
            (o2, o2t, tqh, tkh),
        ):
            nc.vector.tensor_mul(
                AP(ot[:].tensor, 0, [[FREE, 128], [9, FREE // 9], [1, 9]]),
                AP(full, t * FREE, ap_d),
                AP(cen, t * FREE + 4, ap_b),
            )
            # DRAM per channel is contiguous: partition r = 8*ch + rr maps
            # to offset r*CHF + t*FREE, uniform across all 128 partitions.
            eng().dma_start(
                AP(od, g * NCH * OUT_CH + t * FREE, [[CHF, 128], [1, FREE]]),
                AP(ot[:].tensor, 0, [[FREE, 128], [1, FREE]]),
            )

    with tile.TileContext(nc) as tc:
        with (
            tc.tile_pool(name="tin", bufs=2) as tin,
            tc.tile_pool(name="tout", bufs=3) as tout,
        ):
            # Program order: G0 loads, G0 s2s, G1 loads, G1 s2s, then the
            # mul+store streams.  DMA-queue FIFOs see all input movement
            # ahead of the mul-gated stores -> no head-of-line blocking.
            groups = []
            for g in range(NG):
                tk = tin.tile([128, CHF], F16, tag="tk")
                tq = tin.tile([128, CHF], F16, tag="tq")
                do_loads(g, tk, tq)
                do_s2s(tk, tq)
                groups.append((g, tk, tq))
            for g, tk, tq in groups:
                for t in range(NT):
                    o1t = tout.tile([128, FREE], F16, tag="o1t")
                    o2t = tout.tile([128, FREE], F16, tag="o2t")
                    do_mul_store(g, t, tk, tq, o1t, o2t)
    _split_waits(nc)
    return nc


_NC_CACHE = []


def _get_nc():
    if not _NC_CACHE:
        _NC_CACHE.append(_build_program())
    return _NC_CACHE[0]


def _variants(x):
    """[B,C,H,W] -> [BC, 3, HP, W] fp16: dj-shifted, row-padded column
    windows of the zero-padded image."""
    xpad = np.pad(
        np.ascontiguousarray(x, dtype=np.float32).reshape(BC, H, W),
        ((0, 0), (1, 1), (1, 1)),
    )
    v = np.stack([xpad[:, :, j : j + W] for j in range(3)], axis=1)
    return np.ascontiguousarray(v.astype(np.float16))


def make_in_maps(key_map, query_map):
    kv = _variants(key_map)
    qv = _variants(query_map)
    return [
        {
            "kp": kv[m * CPC : (m + 1) * CPC],
            "qp": qv[m * CPC : (m + 1) * CPC],
        }
        for m in range(N_CORES)
    ]


def assemble(results):
    out1 = np.concatenate([results[m]["o1"] for m in range(N_CORES)], axis=0)
    out2 = np.concatenate([results[m]["o2"] for m in range(N_CORES)], axis=0)
    return (
        out1.reshape(B, C, L, 9).astype(np.float32),
        out2.reshape(B, C, L, 9).astype(np.float32),
    )


def kernel(key_map, query_map):
    nc = _get_nc()
    in_maps = make_in_maps(key_map, query_map)
    res = run_bass_kernel_spmd(nc, in_maps, core_ids=list(range(N_CORES)))
    return assemble(res.results)


# revision 15
# speedup vs baseline: 1.2760x; 1.2760x over previous
"""Trainium2 Bass kernel for the sparse-attention (local 3x3 unfold) problem.

Math (per batch-channel (b,c), H=W=128, K=3, pad=1):
  ku = unfold(key)  -> [9, L] raw-flat, reinterpreted [L, 9]
  qu = unfold(query)
  out1 = ku * qu[:, 4:5] ; out2 = ku[:, 4:5] * qu   (as [L, 9] views)

The flat per-channel output index n in [0, 9L) decomposes two ways:
  * n = 128*q + j           (chunk q = one (patch p2=q//128, row i2=q%128)
                             slice: 128 contiguous floats of a dj-shifted,
                             row-padded image variant)
  * n = 9*g + e             (group g shares one stride-9 "center" factor)

Device layout (v2, "fat rows"): channel ch of a tile owns 16 partitions
(r = 16*ch + rr) with FREE = 9216 = 72 chunks per partition, n = 9216*rr + f.
  * FREE % 9 == 0 keeps the stride-9 center-broadcast multiply phase-free
    on every partition (one DVE op covers all 8 channels of a tile).
  * Loads: the (72-chunk partition) x (128-chunk patch) overlap gives 24
    maximal segments per channel; each is ONE contiguous DRAM run of the
    variant image -> one descriptor (2-18 KiB) per (segment, channel).
  * Stores: per-channel DRAM is contiguous with offset r*FREE uniform in
    the partition index -> one dma_start moves a whole tile half
    (128 descriptors x 9 KiB).

dtype: fp16 end-to-end on device (harness tolerance 2e-2 vs ~1.5e-3 fp16
error); host upcasts to fp32.  Halves both HBM read and write traffic.

Sharding: pure data-parallel over the 256 (b,c) channels; 32 per core.
"""

import sys

for _p in ("/opt/trn_rl_repo", "/opt/pypackages"):
    if _p not in sys.path:
        sys.path.insert(0, _p)

import numpy as np

import concourse.bass as bass
import concourse.mybir as mybir
import concourse.tile as tile
from concourse.bass import AP
from concourse.bass_utils import run_bass_kernel_spmd
from concourse.vector_clock import ScopedClock

# ---------------------------------------------------------------------------
# Patch: this container's walrus rejects >1 sync-wait on the Tile tail Drain
# ("Too many sync wait commands").  Spill extra waits onto SP NOPs, which
# execute in program order before the all-engine barrier, preserving the
# "all work done before sem clear" semantics.
# ---------------------------------------------------------------------------


def _drain_and_barrier(self, tick_clock, wait_clock):
    nc = self.nc
    drain_inst = nc.sync.drain()
    wait_clock.add_sem_waits(
        drain_inst.ins, ScopedClock({None: tick_clock.global_clock})
    )
    si = drain_inst.ins.sync_info
    if si is not None and len(si.on_wait) > 1:
        waits = list(si.on_wait)
        drain_inst.ins.sync_info = mybir.SyncInfo(
            on_wait=waits[:1], on_update=list(si.on_update)
        )
        for w in waits[1:]:
            nop = nc.sync.nop(nofuse=True)
            nop.ins.sync_info = mybir.SyncInfo(on_wait=[w], on_update=[])

    nc.all_engine_barrier()
    assert self.sems is not None
    popped = nc._tile_sem_poison_stack.pop()
    assert popped is self._sem_poison
    nc.clear_and_free_semaphores(list(self.sems.allocated().values()))
    nc.all_engine_barrier()


tile.TileContext._drain_and_barrier = _drain_and_barrier


def _split_waits(nc, maxw=1):
    """Walrus here allows only `maxw` sync-waits per instruction: move extra
    waits onto same-engine NOPs inserted immediately before the instruction
    (same engine stream => executes before it)."""
    for fn in nc.m.functions:
        for bb in fn.blocks:
            out = []
            for inst in bb.instructions:
                si = getattr(inst, "sync_info", None)
                if si is not None and len(si.on_wait) > maxw:
                    waits = list(si.on_wait)
                    for w in waits[:-maxw]:
                        nop = mybir.InstNoOp(
                            name=nc.get_next_instruction_name(),
                            bass_nofuse=True,
                        )
                        nop.engine = inst.engine
                        nop.sync_info = mybir.SyncInfo(on_wait=[w], on_update=[])
                        nc.register_instruction(nop)
                        out.append(nop)
                    inst.sync_info = mybir.SyncInfo(
                        on_wait=waits[-maxw:], on_update=list(si.on_update)
                    )
                out.append(inst)
            bb.instructions[:] = out

# ---------------------------------------------------------------------------

F16 = mybir.dt.float16

N_CORES = 8
B, C, H, W = 4, 64, 128, 128
BC = B * C                # 256 channels
CPC = BC // N_CORES       # 32 channels per core
NCH = 16                  # channels per tile (x8 partitions = 128)
NG = CPC // NCH           # channel groups per core
HP = H + 2                # padded rows
VAR = HP * W              # one dj-variant: [130, 128]
IMG = 3 * VAR             # three dj-variants per channel
L = H * W
PPCH = 8                  # partitions per channel
CHF = 18432               # elements per partition per channel (144 chunks)
NT = 8                    # f-sub-tiles per channel group
TCH = 144 // NT           # chunks per sub-tile per partition
FREE = CHF // NT          # tile free width (= 9 * k, phase-free multiply)
OUT_CH = 9 * L            # 147456 = PPCH * CHF
assert FREE % 9 == 0


def _segments():
    """Maximal q-runs per channel where (partition rr = q//144, sub-tile
    t = (q%144)//TCH, patch p2 = q//128) are all constant.  Each is one
    contiguous SBUF run AND one contiguous DRAM run of a dj-variant ->
    one descriptor per channel.  Grouped by sub-tile t."""
    bounds = sorted(set(range(0, 1153, TCH)) | set(range(0, 1153, 128)))
    segs = [[] for _ in range(NT)]
    for qs, qe in zip(bounds[:-1], bounds[1:]):
        rr, p2 = qs // 144, qs // 128
        t = (qs - 144 * rr) // TCH
        di, dj = divmod(p2, 3)
        segs[t].append(
            (
                rr,
                (qs - 144 * rr - TCH * t) * 128,      # f offset in tile
                (qe - qs) * 128,                      # run length (elements)
                dj * VAR + (qs - 128 * p2 + di) * W,  # src offset in IMG
            )
        )
    return segs


_SEGS = _segments()  # 40 segments in 4 sub-tile groups


def _build_program():
    nc = bass.Bass(trn_type="TRN2")
    # k and q fused on a leading [2] axis (and o1/o2 likewise) so one
    # dma_start covers both: 32-descriptor loads / 256-descriptor stores
    # halve the dma_start count and keep all 16 SDMA engines 2 deep.
    kq = nc.dram_tensor("kq", [2, CPC, 3, HP, W], F16, kind="ExternalInput")
    oo = nc.dram_tensor("oo", [2, CPC, OUT_CH], F16, kind="ExternalOutput")
    IN_X = CPC * IMG          # DRAM stride between k and q planes
    OUT_X = CPC * OUT_CH      # DRAM stride between o1 and o2 planes

    # Three dynamic DMA queues (SP-HWDGE, ACT-HWDGE, Pool-SWDGE); strict
    # round-robin keeps every queue fed (prior HW finding: greedy
    # bin-packing clusters DMAs per queue and the per-engine FIFO then
    # serializes them).
    engines = [nc.sync, nc.scalar, nc.gpsimd]
    eng_i = [0]

    def eng():
        e = engines[eng_i[0] % len(engines)]
        eng_i[0] += 1
        return e

    F2 = 2 * FREE  # fused tile free width: k then q (or o1 then o2)

    def do_loads(g, t, ti):
        # 32 descriptors (channel-major x {k,q}) per dma_start, mutually
        # non-contiguous in stream order.  Descriptors are dealt to
        # SDMA-engine slots round-robin from slot 0 and consecutive
        # contiguous descriptors re-aggregate into one packet, so
        # 8-descriptor loads pile onto engines 0-7 (HW-measured: 86%
        # busy vs 39% on engines 8-15); 32 descriptors keep all 16
        # engines 2 deep.
        th = ti[:].tensor
        for rr, f_off, seg_len, src_off in _SEGS[t]:
            eng().dma_start(
                AP(th, rr * F2 + f_off,
                   [[PPCH * F2, NCH], [FREE, 2], [1, seg_len]]),
                AP(kq, g * NCH * IMG + src_off,
                   [[IMG, NCH], [IN_X, 2], [1, seg_len]]),
            )

    def do_mul_store(g, t, ti, ot):
        ith, oth = ti[:].tensor, ot[:].tensor
        ap_o = [[F2, 128], [9, FREE // 9], [1, 9]]
        ap_b = [[F2, 128], [9, FREE // 9], [0, 9]]
        # o1 = k_full * q_center ; o2 = q_full * k_center
        nc.vector.tensor_mul(
            AP(oth, 0, ap_o), AP(ith, 0, ap_o), AP(ith, FREE + 4, ap_b)
        )
        nc.vector.tensor_mul(
            AP(oth, FREE, ap_o), AP(ith, FREE, ap_o), AP(ith, 4, ap_b)
        )
        # DRAM per channel is contiguous: partition r = 8*ch + rr maps
        # to offset r*CHF + t*FREE, uniform across all 128 partitions.
        eng().dma_start(
            AP(oo, g * NCH * OUT_CH + t * FREE,
               [[CHF, 128], [OUT_X, 2], [1, FREE]]),
            AP(oth, 0, [[F2, 128], [FREE, 2], [1, FREE]]),
        )

    with tile.TileContext(nc) as tc:
        with (
            tc.tile_pool(name="tin", bufs=3) as tin,
            tc.tile_pool(name="tout", bufs=3) as tout,
        ):
            # Software pipeline with one-tile lookahead so loads of tile
            # n+1 sit AHEAD of (mul-gated) stores of tile n in each DMA
            # engine's FIFO -> no head-of-line blocking on the loads.
            prev = None
            for g in range(NG):
                for t in range(NT):
                    ti = tin.tile([128, F2], F16, tag="ti")
                    do_loads(g, t, ti)
                    if prev is not None:
                        do_mul_store(*prev)
                    ot = tout.tile([128, F2], F16, tag="ot")
                    prev = (g, t, ti, ot)
            do_mul_store(*prev)
    _split_waits(nc)
    return nc


_NC_CACHE = []


def _get_nc():
    if not _NC_CACHE:
        _NC_CACHE.append(_build_program())
    return _NC_CACHE[0]


def _variants(x):
    """[B,C,H,W] -> [BC, 3, HP, W] fp16: dj-shifted, row-padded column
    windows of the zero-padded image."""
    xpad = np.pad(
        np.ascontiguousarray(x, dtype=np.float32).reshape(BC, H, W),
        ((0, 0), (1, 1), (1, 1)),
    )
    v = np.stack([xpad[:, :, j : j + W] for j in range(3)], axis=1)
    return np.ascontiguousarray(v.astype(np.float16))


def make_in_maps(key_map, query_map):
    kv = _variants(key_map)
    qv = _variants(query_map)
    return [
        {
            "kq": np.ascontiguousarray(
                np.stack([kv[m * CPC : (m + 1) * CPC],
                          qv[m * CPC : (m + 1) * CPC]])
            ),
        }
        for m in range(N_CORES)
    ]


def assemble(results):
    out1 = np.concatenate([results[m]["oo"][0] for m in range(N_CORES)], axis=0)
    out2 = np.concatenate([results[m]["oo"][1] for m in range(N_CORES)], axis=0)
    return (
        out1.reshape(B, C, L, 9).astype(np.float32),
        out2.reshape(B, C, L, 9).astype(np.float32),
    )


def kernel(key_map, query_map):
    nc = _get_nc()
    in_maps = make_in_maps(key_map, query_map)
    res = run_bass_kernel_spmd(nc, in_maps, core_ids=list(range(N_CORES)))
    return assemble(res.results)
